# revision 25
# baseline (speedup 1.0000x reference)
"""Trainium2 Bass kernel for nn_FFMLP (4-layer MLP, hidden=128, relu).

Strategy (pure data parallel, batch sharded 8 ways):
- Feature-major on-chip layout: activations live as [feat, batch] so every
  layer is a single K<=128 matmul with the (tiny, replicated) weight as the
  stationary operand and the activation stream as the moving operand.
- fp16 matmul operands (1 cycle/row on the PE vs 4 for fp32), fp32 PSUM
  accumulation; host rounds inputs/weights to fp16 (unbiased ~2^-11).
- Per-512-column chunk pipeline: L0..L3 outputs each occupy one PSUM bank;
  ReLU + downcast PSUM->SBUF is split between ScalarE and VectorE (the
  structural bottleneck: ~1 elem/cycle/lane each from PSUM).
- L4 (M=16) is packed 4 chunks deep into one PSUM bank via column tiling
  (tile_position=(0,32j)) so the final fp32 evacuation is amortized 4x.
- Host transposes x -> x.T per shard and the [16, B/8] result back.
"""
import sys

if "/opt/trn_rl_repo" not in sys.path:
    sys.path.insert(0, "/opt/trn_rl_repo")

import numpy as np

import concourse.bass as bass
import concourse.mybir as mybir
import concourse.tile as tile

INPUT_DIM = 32
OUTPUT_DIM = 16
HIDDEN = 128
PADDED_OUT = 16
NUM_LAYERS = 4
B = 524288
N_CORES = 8
B_CORE = B // N_CORES  # 65536
CHUNK = 512
N_CHUNKS = B_CORE // CHUNK  # 128
GROUP = 4  # chunks packed per L4 PSUM bank (column tiling)
IN_SLAB = 8  # chunks per input DMA

fp16 = mybir.dt.float16
fp32 = mybir.dt.float32
RELU = mybir.ActivationFunctionType.Relu


def _split_waits(nc, max_waits=1):
    """walrus in this image rejects >1 semaphore wait per instruction on some
    formats; split excess waits onto preceding NOPs on the same engine queue
    (queues are in-order, so semantics are preserved)."""
    n_new = 0
    for bb in nc.main_func.blocks:
        out_list = []
        changed = False
        for ins in bb.instructions:
            si = ins.sync_info
            if si is not None and si.on_wait and len(si.on_wait) > max_waits:
                waits = list(si.on_wait)
                extra, keep = waits[:-max_waits], waits[-max_waits:]
                while extra:
                    chunk, extra = extra[:max_waits], extra[max_waits:]
                    n_new += 1
                    nop = mybir.InstNoOp(name=f"I-waitsplit-{n_new}", ins=[], outs=[])
                    nop.engine = ins.engine
                    nop.sync_info = mybir.SyncInfo(on_wait=chunk, on_update=[])
                    out_list.append(nop)
                ins.sync_info = mybir.SyncInfo(on_wait=keep, on_update=si.on_update)
                changed = True
            out_list.append(ins)
        if changed:
            bb.instructions = out_list
    return n_new


def _dedup_ldweights(nc):
    """Tile emits an explicit InstLdweights before every matmul; weights only
    change at those instructions. Replace an InstLdweights whose key
    (weights AP, tile position/size, perf mode) matches the previous one on
    the PE queue with a NOP carrying the same sync_info — the weight reload
    otherwise costs ~93ns ahead of its matmul."""
    n = 0
    for bb in nc.main_func.blocks:
        il = list(bb.instructions)
        last_key = None
        changed = False
        for idx, ins in enumerate(il):
            if ins.engine != mybir.EngineType.PE:
                continue
            if isinstance(ins, mybir.InstLdweights):
                key = (
                    repr(ins.ins[0]),
                    str(ins.tile_position),
                    str(getattr(ins, "tile_size", None)),
                    str(ins.perf_mode),
                    bool(ins.is_transpose),
                )
                if key == last_key:
                    nop = mybir.InstNoOp(name=ins.name, ins=[], outs=[])
                    nop.engine = ins.engine
                    nop.sync_info = ins.sync_info
                    il[idx] = nop
                    changed = True
                    n += 1
                last_key = key
        if changed:
            bb.instructions = il
    return n


def build(n_chunks=N_CHUNKS):
    nc = bass.Bass()
    n_cols = n_chunks * CHUNK
    # xt2: pair-strip layout — xt2[32*i + f, p*CHUNK + c] = x.T[f, (2p+i)*CHUNK + c]
    # so a pair of chunks feeds two concurrent row-tiled K=32 L0 matmuls.
    xt = nc.declare_dram_parameter(
        "xt", [2 * INPUT_DIM, n_cols // 2], fp16, isOutput=False
    )
    w0 = nc.declare_dram_parameter(
        "w0", [2 * INPUT_DIM, HIDDEN], fp16, isOutput=False
    )
    w1 = nc.declare_dram_parameter("w1", [HIDDEN, HIDDEN], fp16, isOutput=False)
    w2 = nc.declare_dram_parameter("w2", [HIDDEN, HIDDEN], fp16, isOutput=False)
    w3 = nc.declare_dram_parameter("w3", [HIDDEN, HIDDEN], fp16, isOutput=False)
    w4 = nc.declare_dram_parameter("w4", [HIDDEN, PADDED_OUT], fp16, isOutput=False)
    yt = nc.declare_dram_parameter("yt", [PADDED_OUT, n_cols], fp32, isOutput=True)

    with tile.TileContext(nc) as tc:
        with (
            tc.tile_pool(name="wp", bufs=1) as wp,
            tc.tile_pool(name="io", bufs=1) as io,
            tc.tile_pool(name="hp", bufs=1) as hp,
            tc.tile_pool(name="ps", bufs=1, space="PSUM") as ps,
        ):
            w0s = wp.tile([2 * INPUT_DIM, HIDDEN], fp16, tag="w0", name="w0s")
            w1s = wp.tile([HIDDEN, HIDDEN], fp16, tag="w1", name="w1s")
            w2s = wp.tile([HIDDEN, HIDDEN], fp16, tag="w2", name="w2s")
            w3s = wp.tile([HIDDEN, HIDDEN], fp16, tag="w3", name="w3s")
            w4s = wp.tile([HIDDEN, PADDED_OUT], fp16, tag="w4", name="w4s")
            nc.sync.dma_start(out=w0s, in_=w0[:, :])
            nc.sync.dma_start(out=w1s, in_=w1[:, :])
            nc.sync.dma_start(out=w2s, in_=w2[:, :])
            nc.sync.dma_start(out=w3s, in_=w3[:, :])
            nc.sync.dma_start(out=w4s, in_=w4[:, :])

            # Software-pipelined emission over chunk pairs. Per round, the
            # deepest stages are emitted first so adjacent PE-queue matmuls
            # come from different stages/chunks and can stream back-to-back.
            #   stage0(pair p)  @ round 2p  : 2 row-tiled L0 MMs -> l0 pair
            #                                 tile, ACT relu FD=1024 -> h1
            #   stage1(chunk c) @ round c+1 : L1 MM, DVE relu -> h2
            #   stage2(pair p)  @ round 2p+3: 2 L2 MMs -> l0 tile (reuse),
            #                                 ACT relu FD=1024 -> h3
            #   stage3(chunk c) @ round c+4 : L3 MM, DVE relu -> h4
            #   stage4(group g) @ round 4g+8: 4 adjacent col-tiled L4 MMs
            #                                 (concurrent), ACT evac, DMA out
            state = {}  # tiles carried between stages
            PAIR = 2 * CHUNK
            SLAB_PAIRS = IN_SLAB // 2  # pairs per input DMA

            # HAM warm-up: dummy matmuls keep the PE busy while the first
            # input slab lands, so real matmuls start at 2.4 GHz instead of
            # paying the ~3.4us cold window at 1.2 GHz.
            pwarm = ps.tile([HIDDEN, 128], fp32, tag="l4", bufs=1, name="pwarm")
            for _ in range(24):
                nc.tensor.matmul(
                    pwarm[:, :], w1s[:, :], w2s[:, 0:128], start=True, stop=True
                )

            def stage0(p):
                if p % SLAB_PAIRS == 0:
                    npair = min(SLAB_PAIRS, n_chunks // 2 - p)
                    state["xslab", p // SLAB_PAIRS] = xs = io.tile(
                        [2 * INPUT_DIM, npair * CHUNK], fp16,
                        tag="xin", bufs=3, name="xs",
                    )
                    nc.sync.dma_start(
                        out=xs, in_=xt[:, p * CHUNK : (p + npair) * CHUNK]
                    )
                xs = state["xslab", p // SLAB_PAIRS]
                o = (p % SLAB_PAIRS) * CHUNK
                p0 = ps.tile([HIDDEN, PAIR], fp32, tag="l0", bufs=2, name="p0")
                for i in range(2):
                    nc.tensor.matmul(
                        p0[:, i * CHUNK : (i + 1) * CHUNK],
                        w0s[32 * i : 32 * i + INPUT_DIM, :],
                        xs[32 * i : 32 * i + INPUT_DIM, o : o + CHUNK],
                        start=True,
                        stop=True,
                        tile_position=(32 * i, 0),
                    )
                h1 = hp.tile([HIDDEN, PAIR], fp16, tag="h1", bufs=2, name="h1")
                nc.scalar.activation(h1[:, :], p0[:, :], RELU)
                state["h1", p] = h1
                state["p0", p] = p0

            def stage1(p):
                h1 = state.pop(("h1", p))
                p1 = ps.tile([HIDDEN, PAIR], fp32, tag="l1", bufs=1, name="p1")
                for i in range(2):
                    nc.tensor.matmul(
                        p1[:, i * CHUNK : (i + 1) * CHUNK],
                        w1s[:, :],
                        h1[:, i * CHUNK : (i + 1) * CHUNK],
                        start=True, stop=True,
                    )
                h2 = hp.tile([HIDDEN, PAIR], fp16, tag="h2", bufs=3, name="h2")
                nc.vector.tensor_scalar_max(h2[:, :], p1[:, :], 0.0)
                state["h2", p] = h2

            def stage2(p):
                p0 = state.pop(("p0", p))
                h2 = state.pop(("h2", p))
                h3 = hp.tile([HIDDEN, PAIR], fp16, tag="h3", bufs=2, name="h3")
                for i in range(2):
                    nc.tensor.matmul(
                        p0[:, i * CHUNK : (i + 1) * CHUNK],
                        w2s[:, :],
                        h2[:, i * CHUNK : (i + 1) * CHUNK],
                        start=True, stop=True,
                    )
                nc.scalar.activation(h3[:, :], p0[:, :], RELU)
                state["h3", p] = h3

            def stage3(c):
                p = c // 2
                h3 = state[("h3", p)]
                i = c % 2
                p3 = ps.tile([HIDDEN, CHUNK], fp32, tag="l3", bufs=1, name="p3")
                nc.tensor.matmul(
                    p3[:, :], w3s[:, :], h3[:, i * CHUNK : (i + 1) * CHUNK],
                    start=True, stop=True,
                )
                if i == 1:
                    del state[("h3", p)]
                h4 = hp.tile([HIDDEN, CHUNK], fp16, tag="h4", bufs=6, name="h4")
                nc.vector.tensor_scalar_max(h4[:, :], p3[:, :], 0.0)
                state["h4", c] = h4

            def stage4(g):
                p4 = ps.tile([HIDDEN, CHUNK], fp32, tag="l4", bufs=1, name="p4")
                for j in range(GROUP):
                    h4 = state.pop(("h4", 4 * g + j))
                    nc.tensor.matmul(
                        p4[32 * j : 32 * j + PADDED_OUT, :],
                        w4s[:, :],
                        h4[:, :],
                        start=True,
                        stop=True,
                        tile_position=(0, 32 * j),
                    )
                osb = io.tile([HIDDEN, CHUNK], fp32, tag="osb", bufs=3, name="osb")
                nc.scalar.copy(out=osb[:, :], in_=p4[:, :])
                for jj in range(GROUP):
                    cc = 4 * g + jj
                    nc.sync.dma_start(
                        out=yt[:, cc * CHUNK : (cc + 1) * CHUNK],
                        in_=osb[32 * jj : 32 * jj + PADDED_OUT, :],
                    )

            assert n_chunks % 4 == 0
            for r in range(n_chunks + 9):
                if r >= 8 and (r - 8) % 4 == 0 and (r - 8) // 4 < n_chunks // 4:
                    stage4((r - 8) // 4)
                if 0 <= r - 4 < n_chunks:
                    stage3(r - 4)
                if r >= 3 and (r - 3) % 2 == 0 and (r - 3) // 2 < n_chunks // 2:
                    stage2((r - 3) // 2)
                if r >= 1 and (r - 1) % 2 == 0 and (r - 1) // 2 < n_chunks // 2:
                    stage1((r - 1) // 2)
                if r % 2 == 0 and r // 2 < n_chunks // 2:
                    stage0(r // 2)
    _dedup_ldweights(nc)
    _split_waits(nc)
    return nc


def _split_weights(weights):
    ws = []
    off = 0
    ws.append(weights[off : off + HIDDEN * INPUT_DIM].reshape(HIDDEN, INPUT_DIM))
    off += HIDDEN * INPUT_DIM
    for _ in range(NUM_LAYERS - 1):
        ws.append(weights[off : off + HIDDEN * HIDDEN].reshape(HIDDEN, HIDDEN))
        off += HIDDEN * HIDDEN
    ws.append(weights[off : off + PADDED_OUT * HIDDEN].reshape(PADDED_OUT, HIDDEN))
    return ws


_NC_CACHE = {}


def make_in_maps(inputs: np.ndarray, weights: np.ndarray):
    ws = _split_weights(np.asarray(weights, dtype=np.float32))
    # stationary operands are lhsT = [K_in, M_out] = W.T; W0.T is stacked
    # twice for the two row-tiled strips.
    w0t = np.ascontiguousarray(ws[0].T).astype(np.float16)
    wmaps = {
        "w0": np.concatenate([w0t, w0t], axis=0),
        "w1": np.ascontiguousarray(ws[1].T).astype(np.float16),
        "w2": np.ascontiguousarray(ws[2].T).astype(np.float16),
        "w3": np.ascontiguousarray(ws[3].T).astype(np.float16),
        "w4": np.ascontiguousarray(ws[4].T).astype(np.float16),
    }
    in_maps = []
    for i in range(N_CORES):
        xc = inputs[i * B_CORE : (i + 1) * B_CORE]
        xtc = np.ascontiguousarray(xc.T).astype(np.float16)  # [32, B_CORE]
        # pair-strip layout: [64, B_CORE//2]
        xt2 = np.ascontiguousarray(
            xtc.reshape(INPUT_DIM, B_CORE // (2 * CHUNK), 2, CHUNK)
            .transpose(2, 0, 1, 3)
            .reshape(2 * INPUT_DIM, B_CORE // 2)
        )
        in_maps.append({"xt": xt2, **wmaps})
    return in_maps


def kernel(inputs: np.ndarray, weights: np.ndarray) -> np.ndarray:
    from concourse.bass_utils import run_bass_kernel_spmd

    assert inputs.shape == (B, INPUT_DIM), inputs.shape
    in_maps = make_in_maps(inputs, weights)
    if "nc" not in _NC_CACHE:
        _NC_CACHE["nc"] = build()
    nc = _NC_CACHE["nc"]
    res = run_bass_kernel_spmd(nc, in_maps, list(range(N_CORES)))
    outs = [np.ascontiguousarray(r["yt"].T) for r in res.results]
    return np.concatenate(outs, axis=0)[:, :OUTPUT_DIM]


# revision 31
# speedup vs baseline: 1.0432x; 1.0432x over previous
"""Trainium2 Bass kernel for nn_FFMLP (4-layer MLP, hidden=128, relu).

Strategy (pure data parallel, batch sharded 8 ways):
- Feature-major on-chip layout: activations live as [feat, batch] so every
  layer is a single K<=128 matmul with the (tiny, replicated) weight as the
  stationary operand and the activation stream as the moving operand.
- fp16 matmul operands (1 cycle/row on the PE vs 4 for fp32), fp32 PSUM
  accumulation; host rounds inputs/weights to fp16 (unbiased ~2^-11).
- Per-512-column chunk pipeline: L0..L3 outputs each occupy one PSUM bank;
  ReLU + downcast PSUM->SBUF is split between ScalarE and VectorE (the
  structural bottleneck: ~1 elem/cycle/lane each from PSUM).
- L4 (M=16) is packed 4 chunks deep into one PSUM bank via column tiling
  (tile_position=(0,32j)) so the final fp32 evacuation is amortized 4x.
- Host transposes x -> x.T per shard and the [16, B/8] result back.
"""
import sys

if "/opt/trn_rl_repo" not in sys.path:
    sys.path.insert(0, "/opt/trn_rl_repo")

import numpy as np

import concourse.bass as bass
import concourse.mybir as mybir
import concourse.tile as tile

INPUT_DIM = 32
OUTPUT_DIM = 16
HIDDEN = 128
PADDED_OUT = 16
NUM_LAYERS = 4
B = 524288
N_CORES = 8
B_CORE = B // N_CORES  # 65536
CHUNK = 512
N_CHUNKS = B_CORE // CHUNK  # 128
GROUP = 4  # chunks packed per L4 PSUM bank (column tiling)
IN_SLAB = 8  # chunks per input DMA

fp16 = mybir.dt.float16
fp32 = mybir.dt.float32
RELU = mybir.ActivationFunctionType.Relu


def _split_waits(nc, max_waits=1):
    """walrus in this image rejects >1 semaphore wait per instruction on some
    formats; split excess waits onto preceding NOPs on the same engine queue
    (queues are in-order, so semantics are preserved)."""
    n_new = 0
    for bb in nc.main_func.blocks:
        out_list = []
        changed = False
        for ins in bb.instructions:
            si = ins.sync_info
            if si is not None and si.on_wait and len(si.on_wait) > max_waits:
                waits = list(si.on_wait)
                extra, keep = waits[:-max_waits], waits[-max_waits:]
                while extra:
                    chunk, extra = extra[:max_waits], extra[max_waits:]
                    n_new += 1
                    nop = mybir.InstNoOp(name=f"I-waitsplit-{n_new}", ins=[], outs=[])
                    nop.engine = ins.engine
                    nop.sync_info = mybir.SyncInfo(on_wait=chunk, on_update=[])
                    out_list.append(nop)
                ins.sync_info = mybir.SyncInfo(on_wait=keep, on_update=si.on_update)
                changed = True
            out_list.append(ins)
        if changed:
            bb.instructions = out_list
    return n_new


def _dedup_ldweights(nc):
    """Tile emits an explicit InstLdweights before every matmul; weights only
    change at those instructions. Replace an InstLdweights whose key
    (weights AP, tile position/size, perf mode) matches the previous one on
    the PE queue with a NOP carrying the same sync_info — the weight reload
    otherwise costs ~93ns ahead of its matmul."""
    n = 0
    for bb in nc.main_func.blocks:
        il = list(bb.instructions)
        last_key = None
        changed = False
        for idx, ins in enumerate(il):
            if ins.engine != mybir.EngineType.PE:
                continue
            if isinstance(ins, mybir.InstLdweights):
                key = (
                    repr(ins.ins[0]),
                    str(ins.tile_position),
                    str(getattr(ins, "tile_size", None)),
                    str(ins.perf_mode),
                    bool(ins.is_transpose),
                )
                if key == last_key:
                    nop = mybir.InstNoOp(name=ins.name, ins=[], outs=[])
                    nop.engine = ins.engine
                    nop.sync_info = ins.sync_info
                    il[idx] = nop
                    changed = True
                    n += 1
                last_key = key
        if changed:
            bb.instructions = il
    return n


def build(n_chunks=N_CHUNKS):
    nc = bass.Bass()
    n_cols = n_chunks * CHUNK
    # xt2: pair-strip layout — xt2[32*i + f, p*CHUNK + c] = x.T[f, (2p+i)*CHUNK + c]
    # so a pair of chunks feeds two concurrent row-tiled K=32 L0 matmuls.
    xt = nc.declare_dram_parameter(
        "xt", [2 * INPUT_DIM, n_cols // 2], fp16, isOutput=False
    )
    w0 = nc.declare_dram_parameter(
        "w0", [2 * INPUT_DIM, HIDDEN], fp16, isOutput=False
    )
    w1 = nc.declare_dram_parameter("w1", [HIDDEN, HIDDEN], fp16, isOutput=False)
    w2 = nc.declare_dram_parameter("w2", [HIDDEN, HIDDEN], fp16, isOutput=False)
    w3 = nc.declare_dram_parameter("w3", [HIDDEN, HIDDEN], fp16, isOutput=False)
    w4 = nc.declare_dram_parameter("w4", [HIDDEN, PADDED_OUT], fp16, isOutput=False)
    yt = nc.declare_dram_parameter("yt", [PADDED_OUT, n_cols], fp32, isOutput=True)

    with tile.TileContext(nc) as tc:
        with (
            tc.tile_pool(name="wp", bufs=1) as wp,
            tc.tile_pool(name="io", bufs=1) as io,
            tc.tile_pool(name="hp", bufs=1) as hp,
            tc.tile_pool(name="ps", bufs=1, space="PSUM") as ps,
        ):
            w0s = wp.tile([2 * INPUT_DIM, HIDDEN], fp16, tag="w0", name="w0s")
            w1s = wp.tile([HIDDEN, HIDDEN], fp16, tag="w1", name="w1s")
            w2s = wp.tile([HIDDEN, HIDDEN], fp16, tag="w2", name="w2s")
            w3s = wp.tile([HIDDEN, HIDDEN], fp16, tag="w3", name="w3s")
            w4s = wp.tile([HIDDEN, PADDED_OUT], fp16, tag="w4", name="w4s")
            nc.sync.dma_start(out=w0s, in_=w0[:, :])
            nc.sync.dma_start(out=w1s, in_=w1[:, :])
            nc.sync.dma_start(out=w2s, in_=w2[:, :])
            nc.sync.dma_start(out=w3s, in_=w3[:, :])
            nc.sync.dma_start(out=w4s, in_=w4[:, :])

            # Software-pipelined emission over chunk pairs. Per round, the
            # deepest stages are emitted first so adjacent PE-queue matmuls
            # come from different stages/chunks and can stream back-to-back.
            #   stage0(pair p)  @ round 2p  : 2 row-tiled L0 MMs -> l0 pair
            #                                 tile, ACT relu FD=1024 -> h1
            #   stage1(chunk c) @ round c+1 : L1 MM, DVE relu -> h2
            #   stage2(pair p)  @ round 2p+3: 2 L2 MMs -> l0 tile (reuse),
            #                                 ACT relu FD=1024 -> h3
            #   stage3(chunk c) @ round c+4 : L3 MM, DVE relu -> h4
            #   stage4(group g) @ round 4g+8: 4 adjacent col-tiled L4 MMs
            #                                 (concurrent), ACT evac, DMA out
            state = {}  # tiles carried between stages
            PAIR = 2 * CHUNK
            SLAB_PAIRS = IN_SLAB // 2  # pairs per input DMA

            # HAM warm-up: dummy matmuls keep the PE busy while the first
            # input slab lands, so real matmuls start at 2.4 GHz instead of
            # paying the ~3.4us cold window at 1.2 GHz.
            pwarm = ps.tile([HIDDEN, 128], fp32, tag="l4", bufs=1, name="pwarm")
            for _ in range(24):
                nc.tensor.matmul(
                    pwarm[:, :], w1s[:, :], w2s[:, 0:128], start=True, stop=True
                )

            def stage0(p):
                if p % SLAB_PAIRS == 0:
                    npair = min(SLAB_PAIRS, n_chunks // 2 - p)
                    state["xslab", p // SLAB_PAIRS] = xs = io.tile(
                        [2 * INPUT_DIM, npair * CHUNK], fp16,
                        tag="xin", bufs=4, name="xs",
                    )
                    nc.sync.dma_start(
                        out=xs, in_=xt[:, p * CHUNK : (p + npair) * CHUNK]
                    )
                xs = state["xslab", p // SLAB_PAIRS]
                o = (p % SLAB_PAIRS) * CHUNK
                p0 = ps.tile([HIDDEN, PAIR], fp32, tag="l0", bufs=2, name="p0")
                for i in range(2):
                    nc.tensor.matmul(
                        p0[:, i * CHUNK : (i + 1) * CHUNK],
                        w0s[32 * i : 32 * i + INPUT_DIM, :],
                        xs[32 * i : 32 * i + INPUT_DIM, o : o + CHUNK],
                        start=True,
                        stop=True,
                        tile_position=(32 * i, 0),
                    )
                h1 = hp.tile([HIDDEN, PAIR], fp16, tag="h1", bufs=2, name="h1")
                nc.scalar.activation(h1[:, :], p0[:, :], RELU)
                state["h1", p] = h1
                state["p0", p] = p0

            def stage1(c):
                p = c // 2
                h1 = state[("h1", p)]
                i = c % 2
                p1 = ps.tile([HIDDEN, CHUNK], fp32, tag="l1", bufs=2, name="p1")
                nc.tensor.matmul(
                    p1[:, :], w1s[:, :], h1[:, i * CHUNK : (i + 1) * CHUNK],
                    start=True, stop=True,
                )
                if i == 1:
                    del state[("h1", p)]
                h2 = hp.tile([HIDDEN, CHUNK], fp16, tag="h2", bufs=6, name="h2")
                nc.vector.tensor_scalar_max(h2[:, :], p1[:, :], 0.0)
                state["h2", c] = h2

            def stage2(p):
                p0 = state.pop(("p0", p))
                h3 = hp.tile([HIDDEN, PAIR], fp16, tag="h3", bufs=2, name="h3")
                for i in range(2):
                    h2 = state.pop(("h2", 2 * p + i))
                    nc.tensor.matmul(
                        p0[:, i * CHUNK : (i + 1) * CHUNK],
                        w2s[:, :], h2[:, :],
                        start=True, stop=True,
                    )
                nc.scalar.activation(h3[:, :], p0[:, :], RELU)
                state["h3", p] = h3

            def stage3(c):
                p = c // 2
                h3 = state[("h3", p)]
                i = c % 2
                p3 = ps.tile([HIDDEN, CHUNK], fp32, tag="l3", bufs=1, name="p3")
                nc.tensor.matmul(
                    p3[:, :], w3s[:, :], h3[:, i * CHUNK : (i + 1) * CHUNK],
                    start=True, stop=True,
                )
                if i == 1:
                    del state[("h3", p)]
                h4 = hp.tile([HIDDEN, CHUNK], fp16, tag="h4", bufs=8, name="h4")
                nc.vector.tensor_scalar_max(h4[:, :], p3[:, :], 0.0)
                state["h4", c] = h4

            def stage4(g):
                p4 = ps.tile([HIDDEN, CHUNK], fp32, tag="l4", bufs=1, name="p4")
                for j in range(GROUP):
                    h4 = state.pop(("h4", 4 * g + j))
                    nc.tensor.matmul(
                        p4[32 * j : 32 * j + PADDED_OUT, :],
                        w4s[:, :],
                        h4[:, :],
                        start=True,
                        stop=True,
                        tile_position=(0, 32 * j),
                    )
                osb = io.tile([HIDDEN, CHUNK], fp32, tag="osb", bufs=4, name="osb")
                nc.scalar.copy(out=osb[:, :], in_=p4[:, :])
                for jj in range(GROUP):
                    cc = 4 * g + jj
                    nc.sync.dma_start(
                        out=yt[:, cc * CHUNK : (cc + 1) * CHUNK],
                        in_=osb[32 * jj : 32 * jj + PADDED_OUT, :],
                    )

            assert n_chunks % 4 == 0
            for r in range(n_chunks + 9):
                if r >= 8 and (r - 8) % 4 == 0 and (r - 8) // 4 < n_chunks // 4:
                    stage4((r - 8) // 4)
                if 0 <= r - 4 < n_chunks:
                    stage3(r - 4)
                if r >= 3 and (r - 3) % 2 == 0 and (r - 3) // 2 < n_chunks // 2:
                    stage2((r - 3) // 2)
                if 0 <= r - 1 < n_chunks:
                    stage1(r - 1)
                if r % 2 == 0 and r // 2 < n_chunks // 2:
                    stage0(r // 2)
    _dedup_ldweights(nc)
    _split_waits(nc)
    return nc


def _split_weights(weights):
    ws = []
    off = 0
    ws.append(weights[off : off + HIDDEN * INPUT_DIM].reshape(HIDDEN, INPUT_DIM))
    off += HIDDEN * INPUT_DIM
    for _ in range(NUM_LAYERS - 1):
        ws.append(weights[off : off + HIDDEN * HIDDEN].reshape(HIDDEN, HIDDEN))
        off += HIDDEN * HIDDEN
    ws.append(weights[off : off + PADDED_OUT * HIDDEN].reshape(PADDED_OUT, HIDDEN))
    return ws


_NC_CACHE = {}


def make_in_maps(inputs: np.ndarray, weights: np.ndarray):
    ws = _split_weights(np.asarray(weights, dtype=np.float32))
    # stationary operands are lhsT = [K_in, M_out] = W.T; W0.T is stacked
    # twice for the two row-tiled strips.
    w0t = np.ascontiguousarray(ws[0].T).astype(np.float16)
    wmaps = {
        "w0": np.concatenate([w0t, w0t], axis=0),
        "w1": np.ascontiguousarray(ws[1].T).astype(np.float16),
        "w2": np.ascontiguousarray(ws[2].T).astype(np.float16),
        "w3": np.ascontiguousarray(ws[3].T).astype(np.float16),
        "w4": np.ascontiguousarray(ws[4].T).astype(np.float16),
    }
    in_maps = []
    for i in range(N_CORES):
        xc = inputs[i * B_CORE : (i + 1) * B_CORE]
        xtc = np.ascontiguousarray(xc.T).astype(np.float16)  # [32, B_CORE]
        # pair-strip layout: [64, B_CORE//2]
        xt2 = np.ascontiguousarray(
            xtc.reshape(INPUT_DIM, B_CORE // (2 * CHUNK), 2, CHUNK)
            .transpose(2, 0, 1, 3)
            .reshape(2 * INPUT_DIM, B_CORE // 2)
        )
        in_maps.append({"xt": xt2, **wmaps})
    return in_maps


def kernel(inputs: np.ndarray, weights: np.ndarray) -> np.ndarray:
    from concourse.bass_utils import run_bass_kernel_spmd

    assert inputs.shape == (B, INPUT_DIM), inputs.shape
    in_maps = make_in_maps(inputs, weights)
    if "nc" not in _NC_CACHE:
        _NC_CACHE["nc"] = build()
    nc = _NC_CACHE["nc"]
    res = run_bass_kernel_spmd(nc, in_maps, list(range(N_CORES)))
    outs = [np.ascontiguousarray(r["yt"].T) for r in res.results]
    return np.concatenate(outs, axis=0)[:, :OUTPUT_DIM]
